# revision 13
# baseline (speedup 1.0000x reference)
"""Trainium2 Bass kernel for nn_DistanceLoss.

Computes: sum over batch of ||centers[argmax(pred, -1)] - centers[true]|| / 255

Strategy (data-parallel over 8 NeuronCores, B=65536 rows split 8192/core):
  - Stream pred shard through SBUF in 64 tiles of [128 rows, 1000 classes]
    on the SP (sync) HWDGE queue, 8-slot ring buffer.
  - Argmax per row with exactly ONE vector-engine pass over the data:
      * DVE: running-max scan (tensor_tensor_scan, op=max) -> cummax.
      * ACT: idx = sum_t sign(rowmax - cummax[t]) (counts elements strictly
        before the first position attaining the max == jnp.argmax index,
        first-index tie-break included) via one activation with accum_out
        on the otherwise-idle scalar engine.
    This keeps DVE at one pass so the kernel stays HBM-bound (~33MB/core),
    matching the memory target regime.
  - Gather both center pairs with two indirect DMAs from the replicated
    [1000, 2] DRAM table, tiny fused distance epilogue
    sqrt((dx^2+dy^2)/255^2) with row-sum accumulation.
  - Each core emits [128] partial sums; host reduces 8x128 values.

Raw bass blocks with explicit semaphores (no TileContext): walrus's
direct2d pseudo-DMA encodes at most one attached sync-wait, so waits are
issued as separate engine instructions instead.
"""

import sys
from contextlib import ExitStack

import numpy as np

if "/opt/trn_rl_repo" not in sys.path:  # harness-proof import of concourse
    sys.path.insert(0, "/opt/trn_rl_repo")

B = 65536
C = 1000
N_CORES = 8
ROWS_PER_CORE = B // N_CORES          # 8192
P = 128                               # SBUF partitions
T = ROWS_PER_CORE // P                # 64 tiles per core
SLOTS = 8                             # pred ring slots
CMX = 4                               # cummax ring slots

_CACHE = {}


def _build():
    import concourse.bass as bass
    from concourse import mybir

    FP32 = mybir.dt.float32
    U32 = mybir.dt.uint32
    Act = mybir.ActivationFunctionType
    Alu = mybir.AluOpType

    nc = bass.Bass()
    pred_d = nc.declare_dram_parameter("pred", [ROWS_PER_CORE, C], FP32, isOutput=False)
    true_d = nc.declare_dram_parameter("true_idx", [P, T], U32, isOutput=False)
    cent_d = nc.declare_dram_parameter("centers", [C, 2], FP32, isOutput=False)
    out_d = nc.declare_dram_parameter("partial", [P, 1], FP32, isOutput=True)

    with ExitStack() as ctx:
        x_buf = ctx.enter_context(nc.sbuf_tensor("x_buf", [P, SLOTS * C], FP32))
        cmx_buf = ctx.enter_context(nc.sbuf_tensor("cmx_buf", [P, CMX * C], FP32))
        junk = ctx.enter_context(nc.sbuf_tensor("junk", [P, C], FP32))
        idx_f = ctx.enter_context(nc.sbuf_tensor("idx_f", [P, T], FP32))
        idx_u = ctx.enter_context(nc.sbuf_tensor("idx_u", [P, T], U32))
        true_sb = ctx.enter_context(nc.sbuf_tensor("true_sb", [P, T], U32))
        ca = ctx.enter_context(nc.sbuf_tensor("ca", [P, T, 2], FP32))
        cb = ctx.enter_context(nc.sbuf_tensor("cb", [P, T, 2], FP32))
        d2 = ctx.enter_context(nc.sbuf_tensor("d2", [P, T, 2], FP32))
        s2 = ctx.enter_context(nc.sbuf_tensor("s2", [P, T], FP32))
        dist = ctx.enter_context(nc.sbuf_tensor("dist", [P, T], FP32))
        part_sb = ctx.enter_context(nc.sbuf_tensor("part_sb", [P, 1], FP32))

        block = ctx.enter_context(nc.Block())
        s_x = [ctx.enter_context(nc.semaphore(f"s_x{i}")) for i in range(SLOTS)]
        s_scan = ctx.enter_context(nc.semaphore("s_scan"))
        s_act = ctx.enter_context(nc.semaphore("s_act"))
        s_idx = ctx.enter_context(nc.semaphore("s_idx"))
        s_true = ctx.enter_context(nc.semaphore("s_true"))
        s_g = ctx.enter_context(nc.semaphore("s_g"))
        s_eps = ctx.enter_context(nc.semaphore("s_eps"))
        s_fin = ctx.enter_context(nc.semaphore("s_fin"))
        s_out = ctx.enter_context(nc.semaphore("s_out"))

        def xs(t):
            return x_buf[:, (t % SLOTS) * C:(t % SLOTS) * C + C]

        def cs(t):
            return cmx_buf[:, (t % CMX) * C:(t % CMX) * C + C]

        @block.sync
        def _(sp):
            sp.dma_start(out=true_sb[:], in_=true_d[:]).then_inc(s_true, 16)
            for t in range(T):
                if t >= SLOTS:
                    # slot free once its previous tile's scan completed
                    sp.wait_ge(s_scan, t - SLOTS + 1)
                sp.dma_start(out=xs(t), in_=pred_d[t * P:(t + 1) * P, :]).then_inc(
                    s_x[t % SLOTS], 16
                )
            sp.wait_ge(s_fin, 1)
            sp.dma_start(out=out_d[:], in_=part_sb[:]).then_inc(s_out, 16)
            sp.wait_ge(s_out, 16)

        @block.vector
        def _(v):
            for t in range(T):
                v.wait_ge(s_x[t % SLOTS], 16 * (t // SLOTS + 1))
                if t >= CMX:
                    # cmx slot free once its previous tile's sign pass read it
                    v.wait_ge(s_act, t - CMX + 1)
                v.tensor_tensor_scan(
                    out=cs(t),
                    data0=xs(t),
                    data1=xs(t),
                    initial=-1.0e30,
                    op0=Alu.max,
                    op1=Alu.max,
                ).then_inc(s_scan, 1)
            v.wait_ge(s_g, 16 * 2 * T)
            v.tensor_tensor(out=d2[:], in0=ca[:], in1=cb[:], op=Alu.subtract).then_inc(
                s_eps, 1
            )
            v.wait_ge(s_eps, 1)
            v.tensor_tensor(out=d2[:], in0=d2[:], in1=d2[:], op=Alu.mult).then_inc(
                s_eps, 1
            )
            v.wait_ge(s_eps, 2)
            v.tensor_tensor(
                out=s2[:], in0=d2[:, :, 0], in1=d2[:, :, 1], op=Alu.add
            ).then_inc(s_eps, 1)

        @block.scalar
        def _(act):
            for t in range(T):
                act.wait_ge(s_scan, t + 1)
                if t >= 1:
                    # same-engine WAW on junk needs explicit sync (write
                    # buffers can drain out of order)
                    act.wait_ge(s_act, t)
                # idx = sum_j sign(rowmax - cummax[j]) accumulated into col t
                act.activation(
                    out=junk[:],
                    in_=cs(t),
                    func=Act.Sign,
                    bias=cs(t)[:, C - 1:C],
                    scale=-1.0,
                    accum_out=idx_f[:, t:t + 1],
                ).then_inc(s_act, 1)
            act.wait_ge(s_eps, 3)
            act.activation(
                out=dist[:],
                in_=s2[:],
                func=Act.Sqrt,
                scale=1.0 / (255.0 * 255.0),
                accum_out=part_sb[:],
            ).then_inc(s_fin, 1)

        @block.gpsimd
        def _(g):
            # Per-tile [P,1] gathers (production scatter_add pattern). The
            # multi-index-per-partition indirect gather mis-traverses the
            # offset AP on hardware, so one index column per instruction.
            g.wait_ge(s_true, 16)
            GRP = 8
            for grp in range(T // GRP):
                g.wait_ge(s_act, GRP * (grp + 1))
                g.tensor_copy(
                    out=idx_u[:, grp * GRP:(grp + 1) * GRP],
                    in_=idx_f[:, grp * GRP:(grp + 1) * GRP],
                ).then_inc(s_idx, 1)
                g.wait_ge(s_idx, grp + 1)
                for k in range(GRP):
                    t = grp * GRP + k
                    g.indirect_dma_start(
                        out=ca[:, t, :],
                        out_offset=None,
                        in_=cent_d[:],
                        in_offset=bass.IndirectOffsetOnAxis(
                            ap=idx_u[:, t:t + 1], axis=0
                        ),
                    ).then_inc(s_g, 16)
                    g.indirect_dma_start(
                        out=cb[:, t, :],
                        out_offset=None,
                        in_=cent_d[:],
                        in_offset=bass.IndirectOffsetOnAxis(
                            ap=true_sb[:, t:t + 1], axis=0
                        ),
                    ).then_inc(s_g, 16)

    return nc


def _get_nc():
    if "nc" not in _CACHE:
        _CACHE["nc"] = _build()
    return _CACHE["nc"]


def kernel(pred, true, centers):
    from concourse.bass_utils import run_bass_kernel_spmd

    pred = np.ascontiguousarray(np.asarray(pred), dtype=np.float32)
    true_u32 = np.asarray(true).astype(np.uint32)
    centers = np.ascontiguousarray(np.asarray(centers), dtype=np.float32)

    in_maps = []
    for c in range(N_CORES):
        lo = c * ROWS_PER_CORE
        hi = lo + ROWS_PER_CORE
        # [P, T] with [p, t] = true[lo + t*128 + p], matching tile layout
        t_shard = np.ascontiguousarray(true_u32[lo:hi].reshape(T, P).T)
        in_maps.append({
            "pred": pred[lo:hi],
            "true_idx": t_shard,
            "centers": centers,
        })

    res = run_bass_kernel_spmd(_get_nc(), in_maps, list(range(N_CORES))).results
    total = 0.0
    for r in res:
        total += float(np.sum(r["partial"].astype(np.float64)))
    return np.float32(total)


# revision 24
# speedup vs baseline: 1.4856x; 1.4856x over previous
"""Trainium2 Bass kernel for nn_DistanceLoss.

Computes: sum over batch of ||centers[argmax(pred, -1)] - centers[true]|| / 255

Strategy (data-parallel over 8 NeuronCores, B=65536 rows split 8192/core):
  - Stream pred shard through SBUF in 64 tiles of [128 rows, 1000 classes]
    on the SP (sync) HWDGE queue, 8-slot ring buffer.
  - Argmax per row with exactly ONE vector-engine pass over the data:
      * DVE: running-max scan (tensor_tensor_scan, op0=max, op1=bypass).
      * ACT: idx = sum_t sign(rowmax - cummax[t]) (counts elements strictly
        before the first position attaining the max == jnp.argmax index,
        first-index tie-break included) via one activation with accum_out
        on the otherwise-idle scalar engine.
  - Pred-side center lookup: per-tile [P,1] indirect DMA gathers on gpsimd
    (the only gather available in the standard ucode library), pipelined in
    groups of 8 behind the scalar-engine index production so they hide
    under the scan stream. True-side lookup is input-only, so it is
    precomputed on the host and DMA'd in as a [128, 64, 2] input.
  - Tiny fused distance epilogue sqrt((dx^2+dy^2)/255^2) with row-sum
    accumulation; each core emits [128] partial sums; host reduces 8x128.

Raw bass blocks with explicit semaphores (no TileContext): walrus's
direct2d pseudo-DMA encodes at most one attached sync-wait, so waits are
issued as separate engine instructions instead.
"""

import sys
from contextlib import ExitStack

import numpy as np

if "/opt/trn_rl_repo" not in sys.path:  # harness-proof import of concourse
    sys.path.insert(0, "/opt/trn_rl_repo")

B = 65536
C = 1000
N_CORES = 8
ROWS_PER_CORE = B // N_CORES          # 8192
P = 128                               # SBUF partitions
T = ROWS_PER_CORE // P                # 64 tiles per core
SLOTS = 8                             # pred ring slots
CMX = 4                               # cummax ring slots
GRP = 8                               # gather group size (tiles)

_CACHE = {}


def _build():
    import concourse.bass as bass
    from concourse import mybir

    FP32 = mybir.dt.float32
    U32 = mybir.dt.uint32
    Act = mybir.ActivationFunctionType
    Alu = mybir.AluOpType

    nc = bass.Bass()
    pred_d = nc.declare_dram_parameter("pred", [ROWS_PER_CORE, C], FP32, isOutput=False)
    cb_d = nc.declare_dram_parameter("cb_pre", [P, T, 2], FP32, isOutput=False)
    cent_d = nc.declare_dram_parameter("centers", [C, 2], FP32, isOutput=False)
    out_d = nc.declare_dram_parameter("partial", [P, 1], FP32, isOutput=True)

    with ExitStack() as ctx:
        x_buf = ctx.enter_context(nc.sbuf_tensor("x_buf", [P, SLOTS * C], FP32))
        cmx_buf = ctx.enter_context(nc.sbuf_tensor("cmx_buf", [P, CMX * C], FP32))
        junk = ctx.enter_context(nc.sbuf_tensor("junk", [P, C], FP32))
        idx_f = ctx.enter_context(nc.sbuf_tensor("idx_f", [P, T], FP32))
        idx_u = ctx.enter_context(nc.sbuf_tensor("idx_u", [P, T], U32))
        ca = ctx.enter_context(nc.sbuf_tensor("ca", [P, T, 2], FP32))
        cb = ctx.enter_context(nc.sbuf_tensor("cb", [P, T, 2], FP32))
        d2 = ctx.enter_context(nc.sbuf_tensor("d2", [P, T, 2], FP32))
        s2 = ctx.enter_context(nc.sbuf_tensor("s2", [P, T], FP32))
        dist = ctx.enter_context(nc.sbuf_tensor("dist", [P, T], FP32))
        part_sb = ctx.enter_context(nc.sbuf_tensor("part_sb", [P, 1], FP32))

        block = ctx.enter_context(nc.Block())
        s_x = [ctx.enter_context(nc.semaphore(f"s_x{i}")) for i in range(SLOTS)]
        s_scan = ctx.enter_context(nc.semaphore("s_scan"))
        s_act = ctx.enter_context(nc.semaphore("s_act"))
        s_idx = ctx.enter_context(nc.semaphore("s_idx"))
        s_cb = ctx.enter_context(nc.semaphore("s_cb"))
        s_g = ctx.enter_context(nc.semaphore("s_g"))
        s_eps = ctx.enter_context(nc.semaphore("s_eps"))
        s_fin = ctx.enter_context(nc.semaphore("s_fin"))
        s_out = ctx.enter_context(nc.semaphore("s_out"))

        def xs(t):
            return x_buf[:, (t % SLOTS) * C:(t % SLOTS) * C + C]

        def cs(t):
            return cmx_buf[:, (t % CMX) * C:(t % CMX) * C + C]

        @block.sync
        def _(sp):
            sp.dma_start(out=cb[:], in_=cb_d[:]).then_inc(s_cb, 16)
            for t in range(T):
                if t >= SLOTS:
                    # slot free once its previous tile's scan completed
                    sp.wait_ge(s_scan, t - SLOTS + 1)
                sp.dma_start(out=xs(t), in_=pred_d[t * P:(t + 1) * P, :]).then_inc(
                    s_x[t % SLOTS], 16
                )
            sp.wait_ge(s_fin, 1)
            sp.dma_start(out=out_d[:], in_=part_sb[:]).then_inc(s_out, 16)
            sp.wait_ge(s_out, 16)

        @block.vector
        def _(v):
            for t in range(T):
                v.wait_ge(s_x[t % SLOTS], 16 * (t // SLOTS + 1))
                if t >= CMX:
                    # cmx slot free once its previous tile's sign pass read it
                    v.wait_ge(s_act, t - CMX + 1)
                v.tensor_tensor_scan(
                    out=cs(t),
                    data0=xs(t),
                    data1=xs(t),
                    initial=-1.0e30,
                    op0=Alu.max,
                    op1=Alu.bypass,
                ).then_inc(s_scan, 1)
            v.wait_ge(s_g, 16 * T)
            v.wait_ge(s_cb, 16)
            v.tensor_tensor(out=d2[:], in0=ca[:], in1=cb[:], op=Alu.subtract).then_inc(
                s_eps, 1
            )
            v.wait_ge(s_eps, 1)
            v.tensor_tensor(out=d2[:], in0=d2[:], in1=d2[:], op=Alu.mult).then_inc(
                s_eps, 1
            )
            v.wait_ge(s_eps, 2)
            v.tensor_tensor(
                out=s2[:], in0=d2[:, :, 0], in1=d2[:, :, 1], op=Alu.add
            ).then_inc(s_eps, 1)

        @block.scalar
        def _(act):
            for t in range(T):
                act.wait_ge(s_scan, t + 1)
                if t >= 1:
                    # same-engine WAW on junk needs explicit sync (write
                    # buffers can drain out of order)
                    act.wait_ge(s_act, t)
                # idx = sum_j sign(rowmax - cummax[j]) accumulated into col t
                act.activation(
                    out=junk[:],
                    in_=cs(t),
                    func=Act.Sign,
                    bias=cs(t)[:, C - 1:C],
                    scale=-1.0,
                    accum_out=idx_f[:, t:t + 1],
                ).then_inc(s_act, 1)
            act.wait_ge(s_eps, 3)
            act.activation(
                out=dist[:],
                in_=s2[:],
                func=Act.Sqrt,
                scale=1.0 / (255.0 * 255.0),
                accum_out=part_sb[:],
            ).then_inc(s_fin, 1)

        @block.gpsimd
        def _(g):
            # Per-tile [P,1] gathers pipelined in groups behind the sign pass
            for grp in range(T // GRP):
                g.wait_ge(s_act, GRP * (grp + 1))
                g.tensor_copy(
                    out=idx_u[:, grp * GRP:(grp + 1) * GRP],
                    in_=idx_f[:, grp * GRP:(grp + 1) * GRP],
                ).then_inc(s_idx, 1)
                g.wait_ge(s_idx, grp + 1)
                for k in range(GRP):
                    t = grp * GRP + k
                    g.indirect_dma_start(
                        out=ca[:, t, :],
                        out_offset=None,
                        in_=cent_d[:],
                        in_offset=bass.IndirectOffsetOnAxis(
                            ap=idx_u[:, t:t + 1], axis=0
                        ),
                    ).then_inc(s_g, 16)

    return nc


def _get_nc():
    if "nc" not in _CACHE:
        _CACHE["nc"] = _build()
    return _CACHE["nc"]


def _prep_maps(pred, true_u32, centers):
    cb_full = centers[true_u32]  # [B, 2] host-side gather (input-only data)
    in_maps = []
    for c in range(N_CORES):
        lo = c * ROWS_PER_CORE
        hi = lo + ROWS_PER_CORE
        cb_pre = np.ascontiguousarray(
            cb_full[lo:hi].reshape(T, P, 2).transpose(1, 0, 2)
        )
        in_maps.append({
            "pred": pred[lo:hi],
            "cb_pre": cb_pre,
            "centers": centers,
        })
    return in_maps


def kernel(pred, true, centers):
    from concourse.bass_utils import run_bass_kernel_spmd

    pred = np.ascontiguousarray(np.asarray(pred), dtype=np.float32)
    true_u32 = np.asarray(true).astype(np.uint32)
    centers = np.ascontiguousarray(np.asarray(centers), dtype=np.float32)

    in_maps = _prep_maps(pred, true_u32, centers)
    res = run_bass_kernel_spmd(_get_nc(), in_maps, list(range(N_CORES))).results
    total = 0.0
    for r in res:
        total += float(np.sum(r["partial"].astype(np.float64)))
    return np.float32(total)
